# revision 23
# baseline (speedup 1.0000x reference)
"""Trainium2 Bass kernel for the attention-LSTM decoder (nn_Dec_9285719294075).

Strategy (8 NeuronCores, SPMD one program):
  Phase R  : recurrence, data-parallel over batch (4 rows / core).
             Kept feature-major (feature on partitions, batch on free) so
             per-step matmuls use 4-column stationary operands (cheap
             LDWEIGHTS) and stream the weights.
             - Uh.T (+b_uh+b_ws) precomputed on device per batch row.
             - cnn/embedding/bias gate contributions precomputed on host
               into a per-step table GHCB (only hx and atten matmuls stay
               in the loop).
  AllGather: hx for all 40 steps ([512, 160] feature-major per core)
             through DRAM bounce buffers.
  Phase F  : vocab projection, tensor-parallel over the 50k vocab
             (6250 / core, W_fc.T resident in SBUF), batched over all
             T*B = 1280 rows.  Bias via a K=1 ones-matmul into PSUM.
             Per-row max + argmax on-device (DVE max/max_index); host
             combines the 8 shard candidates and concatenates logits.
"""

import os
import sys

import numpy as np

for _p in ("/opt/trn_rl_repo", "/root/.axon_site/_ro/trn_rl_repo"):
    if os.path.isdir(_p) and _p not in sys.path:
        sys.path.insert(0, _p)

import concourse.bacc as bacc
import concourse.bass as bass
import concourse.mybir as mybir
import concourse.tile as tile
from concourse.bass_utils import run_bass_kernel_spmd

F32 = mybir.dt.float32
AF = mybir.ActivationFunctionType
AX = mybir.AxisListType
ALU = mybir.AluOpType

B, S, T = 32, 60, 40
E = 512          # RNN_ENC
D = 256          # ZH_DIMS (embedding)
A = 256          # ATT
H = 512          # ZH_HIDDEN
V = 50000        # ZH_VOC
NCORE = 8
BC = B // NCORE          # 4 batch rows per core
VS = V // NCORE          # 6250 vocab per core
TB = T * BC              # 160
TBF = T * B              # 1280
G4 = 4 * H               # 2048 gates
NVT = (VS + 511) // 512  # 13 vocab tiles (12x512 + 106)
NG = TBF // 128          # 10 fc row groups
TPACK = 8                # t rows packed per partition block in GHCB


def _build():
    nc = bacc.Bacc("TRN2", target_bir_lowering=False, debug=False,
                   num_devices=NCORE)

    def di(name, shape, dtype=F32):
        return nc.dram_tensor(name, list(shape), dtype, kind="ExternalInput")

    rnnT = di("rnnT", (BC, E, S))        # rnn_enc[b].T per local batch row
    rnnN = di("rnnN", (S, BC * E))       # rnn_enc natural, b-blocks on free
    WsT = di("WsT", (H, A))
    WuhT = di("WuhT", (E, A))
    Wv = di("Wv", (A, 1))
    batt = di("batt", (A, 1))            # b_uh + b_ws
    WaT = di("WaT", (E, G4))             # W_ih[:, :512].T
    WhhT = di("WhhT", (H, G4))
    GHCB = di("GHCB", (T, BC, G4))       # per-step gate constants [t, b, (j h)]
    WfcT = di("WfcT", (H, VS))
    bfc = di("bfc", (1, VS))
    ones = di("ones", (1, 128))
    eye = di("eye", (128, 128))

    logits = nc.dram_tensor("logits", [T, B, VS], F32, kind="ExternalOutput")
    maxv = nc.dram_tensor("maxv", [T, B], F32, kind="ExternalOutput")
    amax = nc.dram_tensor("amax", [T, B], mybir.dt.uint32, kind="ExternalOutput")
    hxdbg = (nc.dram_tensor("hxdbg", [H, TB], F32, kind="ExternalOutput")
             if os.environ.get("KERNEL_DEBUG_HX") else None)

    with tile.TileContext(nc) as tc:
        with tc.tile_pool(name="persist", bufs=1) as wp, \
             tc.tile_pool(name="dram", bufs=1, space="DRAM") as dp:
            ones_sb = wp.tile([1, 128], F32, name="ones_sb")
            bfc_sb = wp.tile([1, VS], F32, name="bfc_sb")
            agin = dp.tile([H, TB], F32, name="agin")
            agout = dp.tile([NCORE * H, TB], F32, name="agout", addr_space="Shared")

            nc.sync.dma_start(out=ones_sb[:], in_=ones[:])
            nc.sync.dma_start(out=bfc_sb[:], in_=bfc[:])

            # ================= Phase R =================
            with tc.tile_pool(name="rweights", bufs=1) as rp:
                eye_sb = rp.tile([128, 128], F32, name="eye_sb")
                zcol = rp.tile([128, 4], F32, name="zcol")
                hxT_all = [rp.tile([128, TB], F32, name=f"hxT{k}") for k in range(4)]
                nc.sync.dma_start(out=eye_sb[:], in_=eye[:])
                nc.vector.memset(zcol[:], 0.0)
                rnnT_sb = [rp.tile([128, BC * S], F32, name=f"rnnT{k}") for k in range(4)]
                rnnN_sb = rp.tile([S, BC * E], F32, name="rnnN_sb")
                WsT_sb = [rp.tile([128, A], F32, name=f"WsT{k}") for k in range(4)]
                WuhT_sb = [rp.tile([128, A], F32, name=f"WuhT{k}") for k in range(4)]
                Wv_sb = [rp.tile([128, 1], F32, name=f"Wv{k}") for k in range(2)]
                batt_sb = [rp.tile([128, 1], F32, name=f"batt{k}") for k in range(2)]
                WaT_sb = [rp.tile([128, G4], F32, name=f"WaT{k}") for k in range(4)]
                WhhT_sb = [rp.tile([128, G4], F32, name=f"WhhT{k}") for k in range(4)]
                UhT_sb = [rp.tile([128, BC * S], F32, name=f"UhT{k}") for k in range(2)]

                for kc in range(4):
                    for b in range(BC):
                        nc.sync.dma_start(
                            out=rnnT_sb[kc][:, b * S:(b + 1) * S],
                            in_=rnnT[b, kc * 128:(kc + 1) * 128, :],
                        )
                    nc.sync.dma_start(out=WsT_sb[kc][:], in_=WsT[kc * 128:(kc + 1) * 128, :])
                    nc.sync.dma_start(out=WuhT_sb[kc][:], in_=WuhT[kc * 128:(kc + 1) * 128, :])
                    nc.sync.dma_start(out=WhhT_sb[kc][:], in_=WhhT[kc * 128:(kc + 1) * 128, :])
                    nc.sync.dma_start(out=WaT_sb[kc][:], in_=WaT[kc * 128:(kc + 1) * 128, :])
                for kc in range(2):
                    nc.sync.dma_start(out=Wv_sb[kc][:], in_=Wv[kc * 128:(kc + 1) * 128, :])
                    nc.sync.dma_start(out=batt_sb[kc][:], in_=batt[kc * 128:(kc + 1) * 128, :])
                nc.sync.dma_start(out=rnnN_sb[:], in_=rnnN[:])

                # ---- precompute Uh.T per batch row
                with tc.tile_pool(name="ppre", bufs=2, space="PSUM") as ppre:
                    for b in range(BC):
                        for at in range(2):
                            pu = ppre.tile([128, S], F32, name="pu", tag="ppre")
                            for kc in range(4):
                                nc.tensor.matmul(
                                    pu[:],
                                    lhsT=WuhT_sb[kc][:, at * 128:(at + 1) * 128],
                                    rhs=rnnT_sb[kc][:, b * S:(b + 1) * S],
                                    start=(kc == 0), stop=(kc == 3),
                                )
                            nc.scalar.activation(
                                UhT_sb[at][:, b * S:(b + 1) * S], pu[:],
                                AF.Identity, bias=batt_sb[at][:],
                            )

                # ---- the 40-step recurrence
                with tc.tile_pool(name="work", bufs=2) as sp, \
                     tc.tile_pool(name="cxp", bufs=2) as cxp, \
                     tc.tile_pool(name="pgates", bufs=1, space="PSUM") as pgp, \
                     tc.tile_pool(name="psmall", bufs=3, space="PSUM") as psp:

                    cx_prev = cxp.tile([BC, H], F32, name="cx", tag="cx")
                    nc.vector.memset(cx_prev[:], 0.0)
                    hxT_prev = [zcol[:] for _ in range(4)]

                    for t in range(T):
                        # ws = hx @ W_ws.T   -> [4, 256], then transpose
                        pws = psp.tile([BC, A], F32, name="pws", tag="ps")
                        for kc in range(4):
                            nc.tensor.matmul(pws[:], lhsT=hxT_prev[kc], rhs=WsT_sb[kc][:],
                                             start=(kc == 0), stop=(kc == 3))
                        ws_sb = sp.tile([BC, A], F32, name="ws_sb", tag="ws_sb")
                        nc.vector.tensor_copy(ws_sb[:], pws[:])
                        wsT_sb = sp.tile([128, 2 * BC], F32, name="wsT_sb", tag="wsT_sb")
                        for at in range(2):
                            pt = psp.tile([128, BC], F32, name="pwst", tag="ps")
                            nc.tensor.matmul(pt[:], lhsT=ws_sb[:, at * 128:(at + 1) * 128],
                                             rhs=eye_sb[0:BC, 0:BC], is_transpose=True)
                            nc.vector.tensor_copy(wsT_sb[:, at * BC:(at + 1) * BC], pt[:])

                        # tanh(Uh + ws) with per-partition bias
                        tanhv = sp.tile([128, 2 * BC * S], F32, name="tanhv", tag="tanhv")
                        for at in range(2):
                            for b in range(BC):
                                o = at * BC * S + b * S
                                nc.scalar.activation(
                                    tanhv[:, o:o + S], UhT_sb[at][:, b * S:(b + 1) * S],
                                    AF.Tanh, bias=wsT_sb[:, at * BC + b:at * BC + b + 1],
                                )

                        # score.T [60, 4] = Wv . tanhv
                        pscT = psp.tile([S, BC], F32, name="pscT", tag="ps")
                        for b in range(BC):
                            for at in range(2):
                                o = at * BC * S + b * S
                                nc.tensor.matmul(pscT[:, b:b + 1], lhsT=tanhv[:, o:o + S],
                                                 rhs=Wv_sb[at][:],
                                                 start=(at == 0), stop=(at == 1))
                        scT_sb = sp.tile([S, BC], F32, name="scT_sb", tag="scT_sb")
                        nc.vector.tensor_copy(scT_sb[:], pscT[:])
                        psc = psp.tile([BC, S], F32, name="psc", tag="ps")
                        nc.tensor.matmul(psc[:], lhsT=scT_sb[:], rhs=eye_sb[0:S, 0:S],
                                         is_transpose=True)

                        # softmax over s (free dim)
                        negmax = sp.tile([BC, 1], F32, name="negmax", tag="negmax")
                        nc.vector.tensor_reduce(negmax[:], psc[:], axis=AX.X, op=ALU.max,
                                                negate=True)
                        exp_sb = sp.tile([BC, S], F32, name="exp_sb", tag="exp_sb")
                        sumexp = sp.tile([BC, 1], F32, name="sumexp", tag="sumexp")
                        nc.scalar.activation(exp_sb[:], psc[:], AF.Exp, bias=negmax[:],
                                             accum_out=sumexp[:])
                        rcp = sp.tile([BC, 1], F32, name="rcp", tag="rcp")
                        nc.vector.reciprocal(rcp[:], sumexp[:])
                        alpha = sp.tile([BC, S], F32, name="alpha", tag="alpha")
                        nc.vector.tensor_scalar_mul(alpha[:], exp_sb[:], rcp[:])
                        pat = psp.tile([S, BC], F32, name="pat", tag="ps")
                        nc.tensor.matmul(pat[:], lhsT=alpha[:], rhs=eye_sb[0:BC, 0:BC],
                                         is_transpose=True)
                        alphaT = sp.tile([S, BC], F32, name="alphaT", tag="alphaT")
                        nc.vector.tensor_copy(alphaT[:], pat[:])

                        # atten.T [e, b]: 16 small matmuls, then gather to SBUF
                        attT_sb = sp.tile([128, 4 * BC], F32, name="attT_sb", tag="attT_sb")
                        for kc in range(4):
                            pav = psp.tile([128, BC], F32, name="pav", tag="ps")
                            for b in range(BC):
                                nc.tensor.matmul(
                                    pav[:, b:b + 1],
                                    lhsT=rnnN_sb[:, b * E + kc * 128:b * E + (kc + 1) * 128],
                                    rhs=alphaT[:, b:b + 1],
                                    start=True, stop=True,
                                )
                            nc.vector.tensor_copy(attT_sb[:, kc * BC:(kc + 1) * BC], pav[:])

                        # gates psum [4, 2048], free = (j, h); j: 0=i, 1=f, 2=g, 3=o
                        ghcb_t = sp.tile([BC, G4], F32, name="ghcb_t", tag="ghcb_t")
                        nc.sync.dma_start(out=ghcb_t[:], in_=GHCB[t])
                        pg_t = pgp.tile([BC, G4], F32, name="pg", tag="pg")
                        for j in range(4):
                            reg = pg_t[:, j * 512:(j + 1) * 512]
                            for kc in range(4):
                                nc.tensor.matmul(reg, lhsT=hxT_prev[kc],
                                                 rhs=WhhT_sb[kc][:, j * 512:(j + 1) * 512],
                                                 start=(kc == 0), stop=False)
                            for kc in range(4):
                                nc.tensor.matmul(reg, lhsT=attT_sb[:, kc * BC:(kc + 1) * BC],
                                                 rhs=WaT_sb[kc][:, j * 512:(j + 1) * 512],
                                                 start=False, stop=(kc == 3))

                        # gsum = gates + GHCB[t]
                        gsum = sp.tile([BC, G4], F32, name="gsum", tag="gsum")
                        nc.vector.tensor_add(gsum[:], pg_t[:], ghcb_t[:])

                        S_t = sp.tile([BC, G4], F32, name="S_t", tag="S_t")
                        nc.scalar.activation(S_t[:, 0:1024], gsum[:, 0:1024], AF.Sigmoid)
                        nc.scalar.activation(S_t[:, 1024:1536], gsum[:, 1024:1536], AF.Tanh)
                        nc.scalar.activation(S_t[:, 1536:2048], gsum[:, 1536:2048], AF.Sigmoid)

                        cx2 = cxp.tile([BC, H], F32, name="cx", tag="cx")
                        tmp = sp.tile([BC, H], F32, name="tmp", tag="tmp")
                        nc.vector.tensor_mul(tmp[:], S_t[:, 0:512], S_t[:, 1024:1536])
                        nc.vector.tensor_mul(cx2[:], S_t[:, 512:1024], cx_prev[:])
                        nc.vector.tensor_add(cx2[:], cx2[:], tmp[:])
                        tcx = sp.tile([BC, H], F32, name="tcx", tag="tcx")
                        nc.scalar.activation(tcx[:], cx2[:], AF.Tanh)
                        hx2 = sp.tile([BC, H], F32, name="hx2", tag="hx2")
                        nc.vector.tensor_mul(hx2[:], S_t[:, 1536:2048], tcx[:])

                        for kc in range(4):
                            ph = psp.tile([128, BC], F32, name="ph", tag="ps")
                            nc.tensor.matmul(ph[:], lhsT=hx2[:, kc * 128:(kc + 1) * 128],
                                             rhs=eye_sb[0:BC, 0:BC], is_transpose=True)
                            nc.vector.tensor_copy(hxT_all[kc][:, t * BC:(t + 1) * BC], ph[:])

                        hxT_prev = [hxT_all[kc][:, t * BC:(t + 1) * BC] for kc in range(4)]
                        cx_prev = cx2

                # still inside rweights: ship hx to the DRAM bounce buffer
                for kc in range(4):
                    nc.sync.dma_start(out=agin[kc * 128:(kc + 1) * 128, :], in_=hxT_all[kc][:])
                    if hxdbg is not None:
                        nc.sync.dma_start(out=hxdbg[kc * 128:(kc + 1) * 128, :],
                                          in_=hxT_all[kc][:])

            # ================= AllGather =================
            nc.gpsimd.collective_compute(
                "AllGather", ALU.bypass,
                replica_groups=[list(range(NCORE))],
                ins=[agin.opt()], outs=[agout.opt()],
            )

            # ================= Phase F =================
            with tc.tile_pool(name="fcw", bufs=1) as fp:
                WfcT_sb = [fp.tile([128, VS], F32, name=f"WfcT{k}") for k in range(4)]
                for kc in range(4):
                    hv = VS // 2
                    nc.sync.dma_start(out=WfcT_sb[kc][:, :hv],
                                      in_=WfcT[kc * 128:(kc + 1) * 128, :hv])
                    nc.sync.dma_start(out=WfcT_sb[kc][:, hv:],
                                      in_=WfcT[kc * 128:(kc + 1) * 128, hv:])

                hxf = [fp.tile([128, TBF], F32, name=f"hxf{k}") for k in range(4)]
                ag_v = agout[:].rearrange("(c k p) tb -> k p c tb", c=NCORE, k=4, p=128)
                with tc.tile_pool(name="hxraw", bufs=1) as hrp:
                    hxf_raw = [hrp.tile([128, TBF], F32, name=f"hxfr{k}") for k in range(4)]
                    for kc in range(4):
                        nc.sync.dma_start(
                            out=hxf_raw[kc][:].rearrange("p (c tb) -> p c tb", c=NCORE),
                            in_=ag_v[kc],
                        )
                        # reorder columns (c t b) -> (t c b) so rows are t-major
                        nc.vector.tensor_copy(
                            hxf[kc][:].rearrange("p (t c b) -> p t c b", t=T, c=NCORE),
                            hxf_raw[kc][:].rearrange("p (c t b) -> p t c b", c=NCORE, t=T),
                        )

                with tc.tile_pool(name="stage", bufs=2) as stp, \
                     tc.tile_pool(name="fidx", bufs=2) as fip, \
                     tc.tile_pool(name="pfc", bufs=4, space="PSUM") as pfc:
                    for g in range(NG):
                        st = stp.tile([128, VS], F32, name="st", tag="st")
                        for vt in range(NVT):
                            n = min(512, VS - vt * 512)
                            ps = pfc.tile([128, 512], F32, name="ps_fc", tag="ps_fc")
                            for kc in range(4):
                                nc.tensor.matmul(
                                    ps[:, :n],
                                    lhsT=hxf[kc][:, g * 128:(g + 1) * 128],
                                    rhs=WfcT_sb[kc][:, vt * 512:vt * 512 + n],
                                    start=(kc == 0), stop=False,
                                )
                            nc.tensor.matmul(ps[:, :n], lhsT=ones_sb[:],
                                             rhs=bfc_sb[:, vt * 512:vt * 512 + n],
                                             start=False, stop=True)
                            nc.scalar.copy(st[:, vt * 512:vt * 512 + n], ps[:, :n])

                        mx = fip.tile([128, 8], F32, name="mx", tag="mx")
                        ix = fip.tile([128, 8], mybir.dt.uint32, name="ix", tag="ix")
                        nc.vector.max(out=mx[:], in_=st[:])
                        nc.vector.max_index(ix[:], mx[:], st[:])

                        # NOTE: splitting the SBUF partition dim in a DMA AP
                        # ("(t b) v -> t b v") silently corrupts the transfer;
                        # flatten the DRAM side instead (rows are t-major).
                        lfl = logits[:].rearrange("t b v -> (t b) v")
                        nc.sync.dma_start(out=lfl[g * 128:(g + 1) * 128, :], in_=st[:])
                        mfl = maxv[:].rearrange("t b -> (t b)")
                        nc.sync.dma_start(out=mfl[g * 128:(g + 1) * 128], in_=mx[:, 0:1])
                        afl = amax[:].rearrange("t b -> (t b)")
                        nc.sync.dma_start(out=afl[g * 128:(g + 1) * 128], in_=ix[:, 0:1])
    nc.finalize()
    return nc


_NC_CACHE = None


def _get_program():
    global _NC_CACHE
    if _NC_CACHE is None:
        _NC_CACHE = _build()
    return _NC_CACHE


def _prep_in_maps(rnn_enc, cnn_enc, gtruths, emb, W_ih, b_ih, W_hh, b_hh,
                  W_ws, b_ws, W_uh, b_uh, W_v, b_v, W_fc, b_fc):
    f = np.float32
    rnn_enc = np.asarray(rnn_enc, f)
    cnn_enc = np.asarray(cnn_enc, f)
    gtruths = np.asarray(gtruths)
    emb = np.asarray(emb, f)
    W_ih = np.asarray(W_ih, f); b_ih = np.asarray(b_ih, f)
    W_hh = np.asarray(W_hh, f); b_hh = np.asarray(b_hh, f)
    W_ws = np.asarray(W_ws, f); b_ws = np.asarray(b_ws, f)
    W_uh = np.asarray(W_uh, f); b_uh = np.asarray(b_uh, f)
    W_v = np.asarray(W_v, f)
    W_fc = np.asarray(W_fc, f); b_fc = np.asarray(b_fc, f)

    Wc = W_ih[:, E:E + D]            # cnn part
    Wg = W_ih[:, E + D:]             # embedding part
    bsum = b_ih + b_hh

    WsT = np.ascontiguousarray(W_ws.T)           # [512, 256]
    WuhT = np.ascontiguousarray(W_uh.T)          # [512, 256]
    WaT = np.ascontiguousarray(W_ih[:, :E].T)    # [512, 2048]
    WhhT = np.ascontiguousarray(W_hh.T)          # [512, 2048]
    Wv_c = np.ascontiguousarray(W_v.reshape(A, 1) if W_v.size == A else W_v.T)
    batt = np.ascontiguousarray((b_uh + b_ws).reshape(A, 1))
    ones = np.ones((1, 128), f)
    eyem = np.eye(128, dtype=f)

    g_emb = emb[gtruths]                          # [B, T, D]

    in_maps = []
    for c in range(NCORE):
        b0 = c * BC
        v0 = c * VS
        rnnT = np.ascontiguousarray(rnn_enc[b0:b0 + BC].transpose(0, 2, 1))   # [4, 512, 60]
        rnnN = np.ascontiguousarray(
            rnn_enc[b0:b0 + BC].transpose(1, 0, 2).reshape(S, BC * E))        # [60, 4*512]
        # per-step gate constants: g_t @ Wg.T + cnn @ Wc.T + b_ih + b_hh
        pre = (g_emb[b0:b0 + BC] @ Wg.T + (cnn_enc[b0:b0 + BC] @ Wc.T + bsum)[:, None, :])
        ghcb = np.ascontiguousarray(pre.transpose(1, 0, 2))   # [t, b, (j h)]
        WfcT = np.ascontiguousarray(W_fc[v0:v0 + VS].T)                       # [512, 6250]
        bfc_c = np.ascontiguousarray(b_fc[v0:v0 + VS].reshape(1, VS))
        in_maps.append({
            "rnnT": rnnT, "rnnN": rnnN, "WsT": WsT, "WuhT": WuhT, "Wv": Wv_c,
            "batt": batt, "WaT": WaT, "WhhT": WhhT, "GHCB": ghcb,
            "WfcT": WfcT, "bfc": bfc_c, "ones": ones, "eye": eyem,
        })
    return in_maps


def kernel(rnn_enc, cnn_enc, gtruths, emb, W_ih, b_ih, W_hh, b_hh,
           W_ws, b_ws, W_uh, b_uh, W_v, b_v, W_fc, b_fc,
           ssprob=1, is_train=1, _trace=False):
    nc = _get_program()
    in_maps = _prep_in_maps(rnn_enc, cnn_enc, gtruths, emb, W_ih, b_ih,
                            W_hh, b_hh, W_ws, b_ws, W_uh, b_uh, W_v, b_v,
                            W_fc, b_fc)
    res = run_bass_kernel_spmd(nc, in_maps, list(range(NCORE)), trace=_trace)
    outs = res.results

    logits = np.concatenate([outs[c]["logits"] for c in range(NCORE)], axis=2)
    vals = np.stack([outs[c]["maxv"] for c in range(NCORE)])       # [8, T, B]
    idxs = np.stack([outs[c]["amax"] for c in range(NCORE)])       # [8, T, B]
    best = np.argmax(vals, axis=0)                                 # first max wins
    local = np.take_along_axis(idxs, best[None], axis=0)[0]
    preds = (best * VS + local).astype(np.int32)                   # [T, B]
    out = (logits, np.ascontiguousarray(preds.T))
    if _trace:
        return out, res
    return out
